# revision 33
# baseline (speedup 1.0000x reference)
"""Trainium2 Bass kernel for a 2-layer GCN + sigmoid similarity matrix.

Model (see reference):
    h1 = relu(gcn_conv(x, W1, b1));  h2 = relu(gcn_conv(h1, W2, b2))
    out = sigmoid(h2 @ h2.T)                               # [8192, 8192]

gcn_conv(x, W, b) with self-loops and symmetric deg^{-1/2} norm factorizes:
    h  = x @ W
    out[d] = dinv[d] * sum_s Ahat[s, d] * (dinv[s] * h[s]) + b
where Ahat = edge-count matrix + I and dinv = rsqrt(indeg + 1).

Distribution over 8 NeuronCores (dst-sharded, per the sharding hint):
  - Every core computes hs1 = dinv * (x @ W1) for ALL nodes (cheap, replicated)
  - Ahat is densified per core as the [8192 src, 1024 dst] column shard, stored
    fp8_e4m3 (exact small integer counts) -> 8.4MB resident in SBUF.
  - Aggregation is a PE matmul: aggT[f, d] = sum_s hs[s, f] * Ahat[s, d],
    accumulated over 64 src chunks of 128 (lhsT = hs chunk bf16, rhs = A fp8),
    with even/odd chunks in separate PE column groups (concurrent matmuls).
  - Layer-1 output stays feature-major ([64, 1024] bf16 shard). Each core then
    computes hs2 = dinv * (h1 @ W2) for its own shard only and AllGathers the
    row-major [8192, 64] hs2 table, which feeds layer-2 aggregation directly.
  - h2 shards are AllGathered feature-major into [64, 8192]; each core computes
    its [1024, 8192] block of sigmoid(h2 @ h2.T) (bf16 PE matmul K=64 +
    ScalarE sigmoid from PSUM), written as bf16 and upcast to f32 on the host
    (sigmoid outputs here are ~0.52..0.60; bf16 costs ~1e-3 abs err).

Notes: all TensorEngine operands are bf16/fp8 (fp32 matmul runs as two PE
passes); a tiny first AllGather starts the collectives entry barrier (~45us)
early so it overlaps the input DMAs; constants ride one packed f32 tensor and
the weights ride in the x tensor so the startup issues few DMAs.
"""

import os
import sys

# bass/concourse toolchain location (not a problem-statement file)
for _p in ("/opt/trn_rl_repo", "/root/.axon_site/_ro/trn_rl_repo"):
    if os.path.isdir(_p) and _p not in sys.path:
        sys.path.insert(0, _p)
        break

# A cpu-forced JAX would hide the axon-tunneled NeuronCores this kernel needs.
if os.environ.get("JAX_PLATFORMS", "").strip().lower() in ("cpu",):
    os.environ.pop("JAX_PLATFORMS")

import numpy as np
import ml_dtypes

import concourse.bass as bass
import concourse.bacc as bacc
import concourse.mybir as mybir
from concourse import tile
from concourse.bass_utils import run_bass_kernel_spmd

N = 8192          # nodes
E = 262144        # edges
IN_DIM = 128
HID = 64
CORES = 8
DSH = N // CORES  # dst shard size (1024)
NCH = N // 128    # src chunks of 128 (64)
KSH = DSH // 128  # chunks per shard (8)

# packed f32 const tensor columns: degt | degsh | degb2 | b1 | b2
C_DEGT = 0
C_DEGSH = NCH                 # 64
C_DEGB = NCH + KSH            # 72
C_B1 = C_DEGB + 512           # 584
C_B2 = C_B1 + 1               # 585
C_COLS = C_B2 + 1             # 586
# packed bf16 tensor columns: W1 | W2(padded) | xT
XW_COLS = HID + HID + N

F32 = mybir.dt.float32
BF16 = mybir.dt.bfloat16
FP8 = mybir.dt.float8e4
AF = mybir.ActivationFunctionType
ALU = mybir.AluOpType

_COMPILED = {}


def _build_program():
    nc = bacc.Bacc("TRN2", target_bir_lowering=False, debug=False,
                   num_devices=CORES)

    # ---- I/O ----
    xw_d = nc.dram_tensor("xw", [128, XW_COLS], BF16, kind="ExternalInput")
    A_d = nc.dram_tensor("A", [128, NCH * DSH], FP8, kind="ExternalInput")
    c32_d = nc.dram_tensor("c32", [128, C_COLS], F32, kind="ExternalInput")
    out_d = nc.dram_tensor("out", [DSH, N], BF16, kind="ExternalOutput")

    with tile.TileContext(nc) as tc:
        with tc.tile_pool(name="const", bufs=1) as cpool, \
             tc.tile_pool(name="amat", bufs=1) as apool, \
             tc.tile_pool(name="dram", bufs=1, space="DRAM") as dpool:

            # Tiny first collective: starts the entry barrier + ncfw wakeup
            # immediately. Its input is an uninitialized DRAM tile (values
            # are never used) so the trigger has no data dependency and
            # fires right after the NEFF preamble, ahead of the input DMAs.
            warm_in = dpool.tile([64, 16], BF16)
            warm_out = dpool.tile([CORES * 64, 16], BF16)
            nc.gpsimd.collective_compute(
                "AllGather", ALU.bypass,
                replica_groups=[[2 * g, 2 * g + 1] for g in range(CORES // 2)],
                ins=[warm_in.opt()], outs=[warm_out[0:128, :].opt()])

            # ---- packed constants (one DMA on the ACT HWDGE ring) ----
            c32 = cpool.tile([128, C_COLS], F32)
            nc.scalar.dma_start(c32[:], c32_d[:])
            b1_ap = c32[0:HID, C_B1:C_B1 + 1]
            b2_ap = c32[0:HID, C_B2:C_B2 + 1]

            # x + weights (bf16) on the ACT ring, first piece carries W1/W2.
            xw_sb = cpool.tile([128, XW_COLS], BF16)
            nc.scalar.dma_start(xw_sb[:, 0:1152], xw_d[:, 0:1152])
            for a in range(7):
                sl = slice(1152 + a * 1024, 1152 + (a + 1) * 1024)
                nc.scalar.dma_start(xw_sb[:, sl], xw_d[:, sl])
            W1_ap = xw_sb[:, 0:HID]
            W2_ap = xw_sb[0:HID, HID:2 * HID]

            def xT_chunk(c):
                return xw_sb[:, 2 * HID + c * 128: 2 * HID + (c + 1) * 128]

            # A (fp8, SBUF-resident) on the sync ring, in pieces.
            A_sb = apool.tile([128, NCH * DSH], FP8)
            APIECE = NCH // 8
            for a in range(8):
                sl = slice(a * APIECE * DSH, (a + 1) * APIECE * DSH)
                nc.sync.dma_start(A_sb[:, sl], A_d[:, sl])

            # ---- dinv = rsqrt(deg): fast reciprocal + sqrt + Newton step,
            # one fused pipeline over all packed deg columns ----
            DC = C_B1  # 584 deg columns
            deg_all = c32[:, 0:DC]
            r_ = cpool.tile([128, DC], F32)
            dinv = cpool.tile([128, DC], F32)
            t_ = cpool.tile([128, DC], F32)
            nc.vector.reciprocal_approx_fast(r_[:], deg_all)
            nc.scalar.activation(dinv[:], r_[:], AF.Sqrt)
            # Switch ACT to the sigmoid table set now, during idle time
            # (relu lives in every set), so the first sim tile doesn't pay
            # the ~2.7us table load + drain.
            sigwarm = cpool.tile([64, 16], F32)
            nc.scalar.activation(sigwarm[:], c32[0:HID, 0:16], AF.Sigmoid)
            nc.vector.tensor_mul(t_[:], dinv[:], dinv[:])
            nc.vector.tensor_mul(t_[:], t_[:], deg_all)
            nc.vector.tensor_scalar(t_[:], t_[:], -0.5, 1.5, ALU.mult, ALU.add)
            nc.vector.tensor_mul(dinv[:], dinv[:], t_[:])
            dinvt = dinv[:, C_DEGT:C_DEGT + NCH]       # [128, 64]
            dinvsh = dinv[:, C_DEGSH:C_DEGSH + KSH]    # [128, 8]
            dinvb2 = dinv[:, C_DEGB:C_DEGB + 512]      # [128, 512] (2x64 halves)

            h1T_shard = cpool.tile([HID, DSH], BF16)
            h2T_shard = cpool.tile([HID, DSH], BF16)

            def linear_scaled(hs_sb, n_groups, make_lhsT, W_ap, dinv_ap):
                """hs_sb = dinv * (prev @ W), groups of 8 chunks per psum."""
                with tc.tile_pool(name="ph_psum", bufs=3, space="PSUM") as pp:
                    for g in range(n_groups):
                        ph = pp.tile([128, 8 * HID], F32, tag="ph")
                        for k in range(8):
                            nc.tensor.matmul(
                                ph[:, k * HID:(k + 1) * HID],
                                make_lhsT(g * 8 + k), W_ap,
                                start=True, stop=True)
                        dv = dinv_ap[:, g * 8:(g + 1) * 8]
                        nc.vector.tensor_tensor(
                            hs_sb.rearrange("p (c f) -> p c f", f=HID)
                                 [:, g * 8:(g + 1) * 8, :],
                            ph.rearrange("p (c f) -> p c f", f=HID),
                            dv.unsqueeze(2).broadcast_to((128, 8, HID)),
                            ALU.mult)

            def aggregate(hs_sb, b_ap, hT_out, order=None):
                """hT_out [64, DSH] bf16 = relu(dinv_d * (hs.T @ A) + b).
                The two dst halves accumulate concurrently in separate PE
                column groups (same stationary hs chunk loaded to both), so
                both finish as soon as the last chunk is consumed. `order`
                permutes the (sum-commutative) chunk visit order so chunks
                arriving from a split AllGather can be consumed first."""
                pairs = ([(c, c) for c in range(NCH)] if order is None
                         else order)  # (slot in hs_sb, chunk in A)
                with tc.tile_pool(name="ag_psum", bufs=1, space="PSUM") as gp, \
                     tc.tile_pool(name="ag_tmp", bufs=2) as tp:
                    pg = gp.tile([128, 512], F32, tag="pg")
                    for ci, (s, c) in enumerate(pairs):
                        for h in range(2):
                            nc.tensor.matmul(
                                pg[h * HID:(h + 1) * HID, :],
                                hs_sb[:, s * HID:(s + 1) * HID],
                                A_sb[:, c * DSH + h * 512:
                                     c * DSH + (h + 1) * 512],
                                start=(ci == 0), stop=(ci == NCH - 1),
                                tile_position=(0, h * HID),
                                skip_group_check=True)
                    for h in range(2):
                        tmp = tp.tile([HID, 512], F32, tag="tmp")
                        nc.vector.tensor_mul(tmp[:], pg[h * HID:(h + 1) * HID, :],
                                             dinvb2[h * HID:(h + 1) * HID, :])
                        nc.scalar.activation(hT_out[:, h * 512:(h + 1) * 512],
                                             tmp[:], AF.Relu, bias=b_ap)

            # ---- layer 1: hs1 for ALL nodes (replicated), aggregate shard --
            with tc.tile_pool(name="l1", bufs=1) as l1pool:
                hs1 = l1pool.tile([128, NCH * HID], BF16)
                linear_scaled(hs1, NCH // 8, xT_chunk, W1_ap, dinvt)
                aggregate(hs1, b1_ap, h1T_shard)

            # ---- hs2 for OWN shard, allgather row-major, layer 2 ----------
            with tc.tile_pool(name="l2", bufs=1) as l2pool:
                hs2_sh = l2pool.tile([128, KSH * HID], BF16)
                linear_scaled(hs2_sh, 1,
                              lambda k: h1T_shard[:, k * 128:(k + 1) * 128],
                              W2_ap, dinvsh)
                # AllGather hs2 in two halves (first/last 4 chunks of each
                # shard) so layer-2 aggregation starts on the first half
                # while the second is still in flight. hs2 slots are stored
                # in AG arrival order: slot = hh*32 + r*4 + k for node chunk
                # c = r*8 + hh*4 + k.
                hs2 = l2pool.tile([128, NCH * HID], BF16)
                for hh in range(2):
                    agin = dpool.tile([DSH // 2, HID], BF16, name=f"ag2in{hh}")
                    agout = dpool.tile([N // 2, HID], BF16,
                                       addr_space="Shared", name=f"ag2out{hh}")
                    nc.gpsimd.dma_start(
                        agin.rearrange("(k p) f -> p k f", p=128),
                        hs2_sh.rearrange("p (k f) -> p k f", f=HID)
                             [:, hh * 4:(hh + 1) * 4, :])
                    nc.gpsimd.collective_compute(
                        "AllGather", ALU.bypass,
                        replica_groups=[list(range(CORES))],
                        ins=[agin.opt()], outs=[agout.opt()])
                    nc.sync.dma_start(
                        hs2[:, hh * 2048:(hh + 1) * 2048]
                            .rearrange("p (q f) -> p q f", f=HID),
                        agout.rearrange("(q p) f -> p q f", p=128))
                order = [(hh * 32 + r * 4 + k, r * 8 + hh * 4 + k)
                         for hh in range(2) for r in range(CORES)
                         for k in range(4)]
                aggregate(hs2, b2_ap, h2T_shard, order=order)

            # ---- allgather h2 feature-major (two halves), sim + sigmoid ---
            # h2T is duplicated onto partitions 64:128 so pairs of j-tiles
            # run as concurrent K=64 matmuls in separate PE row groups.
            with tc.tile_pool(name="sim", bufs=1) as spool, \
                 tc.tile_pool(name="sim_psum", bufs=2, space="PSUM") as sp, \
                 tc.tile_pool(name="stage", bufs=4) as stpool:
                sh_dup = spool.tile([128, DSH], BF16)
                nc.scalar.dma_start(sh_dup[0:HID, :], h2T_shard[:])
                nc.scalar.dma_start(sh_dup[HID:128, :], h2T_shard[:])
                # free-dim layout of h2T_dup: pass p block at p*4096, then
                # rank r strip of 512 (= h2 cols r*1024 + p*512 + [0, 512))
                h2T_dup = spool.tile([128, N], BF16)
                for p in range(2):
                    agin = dpool.tile([HID, 512], BF16, name=f"ag3in{p}")
                    agout = dpool.tile([CORES * HID, 512], BF16,
                                       addr_space="Shared", name=f"ag3out{p}")
                    nc.gpsimd.dma_start(agin[:],
                                        h2T_shard[:, p * 512:(p + 1) * 512])
                    nc.gpsimd.collective_compute(
                        "AllGather", ALU.bypass,
                        replica_groups=[list(range(CORES))],
                        ins=[agin.opt()], outs=[agout.opt()])
                    src = agout.rearrange("(r f) j -> f r j", f=HID)
                    blk = h2T_dup[:, p * 4096:(p + 1) * 4096]
                    # quarter-loads, low rank-halves first: the first sim
                    # matmuls of this pass need only ranks 0-3
                    for rh in range(2):
                        for g in range(2):
                            nc.scalar.dma_start(
                                blk[g * HID:(g + 1) * HID,
                                    rh * 2048:(rh + 1) * 2048]
                                .rearrange("f (r j) -> f r j", j=512),
                                src[:, rh * 4:(rh + 1) * 4, :])

                out4 = out_d.rearrange("m (r p j) -> m r p j", p=2, j=512)
                for p in range(2):
                    for m in range(DSH // 128):
                        for rq in range(2):
                            st = stpool.tile([128, 2048], BF16, tag="st")
                            ps = sp.tile([128, 2048], F32, tag="ps")
                            for rr in range(4):
                                g = (rr % 2) * HID
                                nc.tensor.matmul(
                                    ps[:, rr * 512:(rr + 1) * 512],
                                    sh_dup[g:g + HID, m * 128:(m + 1) * 128],
                                    h2T_dup[g:g + HID,
                                            p * 4096 + (rq * 4 + rr) * 512:
                                            p * 4096 + (rq * 4 + rr + 1) * 512],
                                    start=True, stop=True,
                                    tile_position=(g, 0),
                                    skip_group_check=True)
                            nc.scalar.activation(st[:], ps[:], AF.Sigmoid)
                            nc.sync.dma_start(
                                out4[m * 128:(m + 1) * 128,
                                     rq * 4:(rq + 1) * 4, p, :],
                                st.rearrange("m (r j) -> m r j", j=512))

    nc.compile()
    return nc


def _get_program():
    if "nc" not in _COMPILED:
        _COMPILED["nc"] = _build_program()
    return _COMPILED["nc"]


def _prep_inputs(x, edge_index, W1, b1, W2, b2):
    x = np.asarray(x, np.float32)
    ei = np.asarray(edge_index)
    src = ei[0].astype(np.int64)
    dst = ei[1].astype(np.int64)

    deg = (np.bincount(dst, minlength=N) + 1).astype(np.float32)
    degt = np.ascontiguousarray(deg.reshape(NCH, 128).T)           # [128, 64]

    xw = np.zeros((128, XW_COLS), dtype=ml_dtypes.bfloat16)
    xw[:, 0:HID] = np.asarray(W1, np.float32).astype(ml_dtypes.bfloat16)
    xw[0:HID, HID:2 * HID] = (
        np.asarray(W2, np.float32).astype(ml_dtypes.bfloat16))
    xw[:, 2 * HID:] = x.T.astype(ml_dtypes.bfloat16)

    b1c = np.asarray(b1, np.float32).reshape(HID)
    b2c = np.asarray(b2, np.float32).reshape(HID)

    in_maps = []
    for i in range(CORES):
        lo = i * DSH
        sel = (dst >= lo) & (dst < lo + DSH)
        flat = src[sel] * DSH + (dst[sel] - lo)
        cnt = np.bincount(flat, minlength=N * DSH).reshape(N, DSH)
        cnt[np.arange(lo, lo + DSH), np.arange(DSH)] += 1          # + I shard
        # SBUF layout: partition p holds src rows {c*128+p}, free = c*DSH + d
        A8 = np.ascontiguousarray(
            cnt.reshape(NCH, 128, DSH).transpose(1, 0, 2)
        ).astype(ml_dtypes.float8_e4m3).reshape(128, NCH * DSH)

        c32 = np.zeros((128, C_COLS), dtype=np.float32)
        c32[:, C_DEGT:C_DEGT + NCH] = degt
        c32[:, C_DEGSH:C_DEGSH + KSH] = deg[lo:lo + DSH].reshape(KSH, 128).T
        degb = np.broadcast_to(deg[lo:lo + DSH][None, :], (HID, DSH))
        c32[:, C_DEGB:C_DEGB + 512] = (
            degb.reshape(HID, 2, 512).transpose(1, 0, 2).reshape(128, 512))
        c32[0:HID, C_B1] = b1c
        c32[0:HID, C_B2] = b2c
        # rsqrt pipeline runs over every deg column; keep the b columns out
        # of it but the whole c32 tile must be finite for the Newton step.
        in_maps.append({"xw": xw, "A": A8, "c32": c32})
    return in_maps


def kernel(x, edge_index, W1, b1, W2, b2, _trace=False, _trace_kwargs=None):
    nc = _get_program()
    in_maps = _prep_inputs(x, edge_index, W1, b1, W2, b2)
    res = run_bass_kernel_spmd(nc, in_maps, core_ids=list(range(CORES)),
                               trace=_trace, **(_trace_kwargs or {}))
    out = np.concatenate([res.results[i]["out"] for i in range(CORES)], axis=0)
    if _trace:
        kernel._last_results = res
    return out.astype(np.float32)


# revision 35
# speedup vs baseline: 2.5595x; 2.5595x over previous
"""Trainium2 Bass kernel for a 2-layer GCN + sigmoid similarity matrix.

Model (see reference):
    h1 = relu(gcn_conv(x, W1, b1));  h2 = relu(gcn_conv(h1, W2, b2))
    out = sigmoid(h2 @ h2.T)                               # [8192, 8192]

gcn_conv(x, W, b) with self-loops and symmetric deg^{-1/2} norm factorizes:
    h  = x @ W
    out[d] = dinv[d] * sum_s Ahat[s, d] * (dinv[s] * h[s]) + b
where Ahat = edge-count matrix + I and dinv = rsqrt(indeg + 1).

Distribution over 8 NeuronCores (dst-sharded, per the sharding hint):
  - Every core computes hs1 = dinv * (x @ W1) for ALL nodes (cheap, replicated)
  - Ahat is densified per core as the [8192 src, 1024 dst] column shard, stored
    fp8_e4m3 (exact small integer counts) -> 8.4MB resident in SBUF.
  - Aggregation is a PE matmul: aggT[f, d] = sum_s hs[s, f] * Ahat[s, d],
    accumulated over 64 src chunks of 128 (lhsT = hs chunk bf16, rhs = A fp8),
    with even/odd chunks in separate PE column groups (concurrent matmuls).
  - Layer-1 output stays feature-major ([64, 1024] bf16 shard). Each core then
    computes hs2 = dinv * (h1 @ W2) for its own shard only and AllGathers the
    row-major [8192, 64] hs2 table, which feeds layer-2 aggregation directly.
  - h2 shards are AllGathered feature-major into [64, 8192]; each core computes
    its [1024, 8192] block of sigmoid(h2 @ h2.T) (bf16 PE matmul K=64 +
    ScalarE sigmoid from PSUM), written as bf16 and upcast to f32 on the host
    (sigmoid outputs here are ~0.52..0.60; bf16 costs ~1e-3 abs err).

Notes: all TensorEngine operands are bf16/fp8 (fp32 matmul runs as two PE
passes); a tiny first AllGather starts the collectives entry barrier (~45us)
early so it overlaps the input DMAs; constants ride one packed f32 tensor and
the weights ride in the x tensor so the startup issues few DMAs.
"""

import os
import sys

# bass/concourse toolchain location (not a problem-statement file)
for _p in ("/opt/trn_rl_repo", "/root/.axon_site/_ro/trn_rl_repo"):
    if os.path.isdir(_p) and _p not in sys.path:
        sys.path.insert(0, _p)
        break

# A cpu-forced JAX would hide the axon-tunneled NeuronCores this kernel needs.
if os.environ.get("JAX_PLATFORMS", "").strip().lower() in ("cpu",):
    os.environ.pop("JAX_PLATFORMS")

import numpy as np
import ml_dtypes

import concourse.bass as bass
import concourse.bacc as bacc
import concourse.mybir as mybir
from concourse import tile
from concourse.bass_utils import run_bass_kernel_spmd

N = 8192          # nodes
E = 262144        # edges
IN_DIM = 128
HID = 64
CORES = 8
DSH = N // CORES  # dst shard size (1024)
NCH = N // 128    # src chunks of 128 (64)
KSH = DSH // 128  # chunks per shard (8)

# packed f32 const tensor columns: degt | degsh | degb2 | b1 | b2
C_DEGT = 0
C_DEGSH = NCH                 # 64
C_DEGB = NCH + KSH            # 72
C_B1 = C_DEGB + 512           # 584
C_B2 = C_B1 + 1               # 585
C_COLS = C_B2 + 1             # 586
# packed bf16 tensor columns: W1 | W2(padded) | xT
XW_COLS = HID + HID + N

F32 = mybir.dt.float32
BF16 = mybir.dt.bfloat16
FP8 = mybir.dt.float8e4
AF = mybir.ActivationFunctionType
ALU = mybir.AluOpType

_COMPILED = {}


def _build_program():
    nc = bacc.Bacc("TRN2", target_bir_lowering=False, debug=False,
                   num_devices=CORES)

    # ---- I/O ----
    xw_d = nc.dram_tensor("xw", [128, XW_COLS], BF16, kind="ExternalInput")
    A_d = nc.dram_tensor("A", [128, NCH * DSH], FP8, kind="ExternalInput")
    c32_d = nc.dram_tensor("c32", [128, C_COLS], F32, kind="ExternalInput")
    out_d = nc.dram_tensor("out", [DSH, N], BF16, kind="ExternalOutput")

    with tile.TileContext(nc) as tc:
        with tc.tile_pool(name="const", bufs=1) as cpool, \
             tc.tile_pool(name="amat", bufs=1) as apool, \
             tc.tile_pool(name="dram", bufs=1, space="DRAM") as dpool:

            # Tiny first collective: starts the entry barrier + ncfw wakeup
            # immediately. Its input is an uninitialized DRAM tile (values
            # are never used) so the trigger has no data dependency and
            # fires right after the NEFF preamble, ahead of the input DMAs.
            warm_in = dpool.tile([64, 16], BF16)
            warm_out = dpool.tile([CORES * 64, 16], BF16)
            nc.gpsimd.collective_compute(
                "AllGather", ALU.bypass,
                replica_groups=[[2 * g, 2 * g + 1] for g in range(CORES // 2)],
                ins=[warm_in.opt()], outs=[warm_out[0:128, :].opt()])

            # ---- packed constants (one DMA on the ACT HWDGE ring) ----
            c32 = cpool.tile([128, C_COLS], F32)
            nc.scalar.dma_start(c32[:], c32_d[:])
            b1_ap = c32[0:HID, C_B1:C_B1 + 1]
            b2_ap = c32[0:HID, C_B2:C_B2 + 1]

            # x + weights (bf16) on the ACT ring, first piece carries W1/W2.
            xw_sb = cpool.tile([128, XW_COLS], BF16)
            nc.scalar.dma_start(xw_sb[:, 0:1152], xw_d[:, 0:1152])
            for a in range(7):
                sl = slice(1152 + a * 1024, 1152 + (a + 1) * 1024)
                nc.scalar.dma_start(xw_sb[:, sl], xw_d[:, sl])
            W1_ap = xw_sb[:, 0:HID]
            W2_ap = xw_sb[0:HID, HID:2 * HID]

            def xT_chunk(c):
                return xw_sb[:, 2 * HID + c * 128: 2 * HID + (c + 1) * 128]

            # A (fp8, SBUF-resident) on the sync ring, in pieces.
            A_sb = apool.tile([128, NCH * DSH], FP8)
            APIECE = NCH // 8
            for a in range(8):
                sl = slice(a * APIECE * DSH, (a + 1) * APIECE * DSH)
                nc.sync.dma_start(A_sb[:, sl], A_d[:, sl])

            # ---- dinv = rsqrt(deg): fast reciprocal + sqrt + Newton step,
            # one fused pipeline over all packed deg columns ----
            DC = C_B1  # 584 deg columns
            deg_all = c32[:, 0:DC]
            r_ = cpool.tile([128, DC], F32)
            dinv = cpool.tile([128, DC], F32)
            t_ = cpool.tile([128, DC], F32)
            nc.vector.reciprocal_approx_fast(r_[:], deg_all)
            nc.scalar.activation(dinv[:], r_[:], AF.Sqrt)
            nc.vector.tensor_mul(t_[:], dinv[:], dinv[:])
            nc.vector.tensor_mul(t_[:], t_[:], deg_all)
            nc.vector.tensor_scalar(t_[:], t_[:], -0.5, 1.5, ALU.mult, ALU.add)
            nc.vector.tensor_mul(dinv[:], dinv[:], t_[:])
            dinvt = dinv[:, C_DEGT:C_DEGT + NCH]       # [128, 64]
            dinvsh = dinv[:, C_DEGSH:C_DEGSH + KSH]    # [128, 8]
            dinvb2 = dinv[:, C_DEGB:C_DEGB + 512]      # [128, 512] (2x64 halves)

            h1T_shard = cpool.tile([HID, DSH], BF16)
            h2T_shard = cpool.tile([HID, DSH], BF16)

            def linear_scaled(hs_sb, n_groups, make_lhsT, W_ap, dinv_ap):
                """hs_sb = dinv * (prev @ W), groups of 8 chunks per psum."""
                with tc.tile_pool(name="ph_psum", bufs=3, space="PSUM") as pp:
                    for g in range(n_groups):
                        ph = pp.tile([128, 8 * HID], F32, tag="ph")
                        for k in range(8):
                            nc.tensor.matmul(
                                ph[:, k * HID:(k + 1) * HID],
                                make_lhsT(g * 8 + k), W_ap,
                                start=True, stop=True)
                        dv = dinv_ap[:, g * 8:(g + 1) * 8]
                        nc.vector.tensor_tensor(
                            hs_sb.rearrange("p (c f) -> p c f", f=HID)
                                 [:, g * 8:(g + 1) * 8, :],
                            ph.rearrange("p (c f) -> p c f", f=HID),
                            dv.unsqueeze(2).broadcast_to((128, 8, HID)),
                            ALU.mult)

            def aggregate(hs_sb, b_ap, hT_out, order=None):
                """hT_out [64, DSH] bf16 = relu(dinv_d * (hs.T @ A) + b).
                The two dst halves accumulate concurrently in separate PE
                column groups (same stationary hs chunk loaded to both), so
                both finish as soon as the last chunk is consumed. `order`
                permutes the (sum-commutative) chunk visit order so chunks
                arriving from a split AllGather can be consumed first."""
                pairs = ([(c, c) for c in range(NCH)] if order is None
                         else order)  # (slot in hs_sb, chunk in A)
                with tc.tile_pool(name="ag_psum", bufs=1, space="PSUM") as gp, \
                     tc.tile_pool(name="ag_tmp", bufs=2) as tp:
                    pg = gp.tile([128, 512], F32, tag="pg")
                    for ci, (s, c) in enumerate(pairs):
                        for h in range(2):
                            nc.tensor.matmul(
                                pg[h * HID:(h + 1) * HID, :],
                                hs_sb[:, s * HID:(s + 1) * HID],
                                A_sb[:, c * DSH + h * 512:
                                     c * DSH + (h + 1) * 512],
                                start=(ci == 0), stop=(ci == NCH - 1),
                                tile_position=(0, h * HID),
                                skip_group_check=True)
                    for h in range(2):
                        tmp = tp.tile([HID, 512], F32, tag="tmp")
                        nc.vector.tensor_mul(tmp[:], pg[h * HID:(h + 1) * HID, :],
                                             dinvb2[h * HID:(h + 1) * HID, :])
                        nc.scalar.activation(hT_out[:, h * 512:(h + 1) * 512],
                                             tmp[:], AF.Relu, bias=b_ap)

            # ---- layer 1: hs1 for ALL nodes (replicated), aggregate shard --
            with tc.tile_pool(name="l1", bufs=1) as l1pool:
                hs1 = l1pool.tile([128, NCH * HID], BF16)
                linear_scaled(hs1, NCH // 8, xT_chunk, W1_ap, dinvt)
                aggregate(hs1, b1_ap, h1T_shard)

            # ---- hs2 for OWN shard, allgather row-major, layer 2 ----------
            with tc.tile_pool(name="l2", bufs=1) as l2pool:
                hs2_sh = l2pool.tile([128, KSH * HID], BF16)
                linear_scaled(hs2_sh, 1,
                              lambda k: h1T_shard[:, k * 128:(k + 1) * 128],
                              W2_ap, dinvsh)
                # AllGather hs2 in two halves (first/last 4 chunks of each
                # shard) so layer-2 aggregation starts on the first half
                # while the second is still in flight. hs2 slots are stored
                # in AG arrival order: slot = hh*32 + r*4 + k for node chunk
                # c = r*8 + hh*4 + k.
                hs2 = l2pool.tile([128, NCH * HID], BF16)
                for hh in range(2):
                    agin = dpool.tile([DSH // 2, HID], BF16, name=f"ag2in{hh}")
                    agout = dpool.tile([N // 2, HID], BF16,
                                       addr_space="Shared", name=f"ag2out{hh}")
                    nc.gpsimd.dma_start(
                        agin.rearrange("(k p) f -> p k f", p=128),
                        hs2_sh.rearrange("p (k f) -> p k f", f=HID)
                             [:, hh * 4:(hh + 1) * 4, :])
                    nc.gpsimd.collective_compute(
                        "AllGather", ALU.bypass,
                        replica_groups=[list(range(CORES))],
                        ins=[agin.opt()], outs=[agout.opt()])
                    nc.sync.dma_start(
                        hs2[:, hh * 2048:(hh + 1) * 2048]
                            .rearrange("p (q f) -> p q f", f=HID),
                        agout.rearrange("(q p) f -> p q f", p=128))
                order = [(hh * 32 + r * 4 + k, r * 8 + hh * 4 + k)
                         for hh in range(2) for r in range(CORES)
                         for k in range(4)]
                aggregate(hs2, b2_ap, h2T_shard, order=order)

            # ---- allgather h2 feature-major (two halves), sim + sigmoid ---
            # h2T is duplicated onto partitions 64:128 so pairs of j-tiles
            # run as concurrent K=64 matmuls in separate PE row groups.
            with tc.tile_pool(name="sim", bufs=1) as spool, \
                 tc.tile_pool(name="sim_psum", bufs=2, space="PSUM") as sp, \
                 tc.tile_pool(name="stage", bufs=4) as stpool:
                sh_dup = spool.tile([128, DSH], BF16)
                nc.scalar.dma_start(sh_dup[0:HID, :], h2T_shard[:])
                nc.scalar.dma_start(sh_dup[HID:128, :], h2T_shard[:])
                # free-dim layout of h2T_dup: pass p block at p*4096, then
                # rank r strip of 512 (= h2 cols r*1024 + p*512 + [0, 512))
                h2T_dup = spool.tile([128, N], BF16)
                for p in range(2):
                    agin = dpool.tile([HID, 512], BF16, name=f"ag3in{p}")
                    agout = dpool.tile([CORES * HID, 512], BF16,
                                       addr_space="Shared", name=f"ag3out{p}")
                    nc.gpsimd.dma_start(agin[:],
                                        h2T_shard[:, p * 512:(p + 1) * 512])
                    nc.gpsimd.collective_compute(
                        "AllGather", ALU.bypass,
                        replica_groups=[list(range(CORES))],
                        ins=[agin.opt()], outs=[agout.opt()])
                    src = agout.rearrange("(r f) j -> f r j", f=HID)
                    blk = h2T_dup[:, p * 4096:(p + 1) * 4096]
                    nc.scalar.dma_start(
                        blk[0:HID, :].rearrange("f (r j) -> f r j", j=512), src)
                    nc.scalar.dma_start(
                        blk[HID:128, :].rearrange("f (r j) -> f r j", j=512),
                        src)

                out4 = out_d.rearrange("m (r p j) -> m r p j", p=2, j=512)
                for p in range(2):
                    for m in range(DSH // 128):
                        for rq in range(2):
                            st = stpool.tile([128, 2048], BF16, tag="st")
                            ps = sp.tile([128, 2048], F32, tag="ps")
                            for rr in range(4):
                                g = (rr % 2) * HID
                                nc.tensor.matmul(
                                    ps[:, rr * 512:(rr + 1) * 512],
                                    sh_dup[g:g + HID, m * 128:(m + 1) * 128],
                                    h2T_dup[g:g + HID,
                                            p * 4096 + (rq * 4 + rr) * 512:
                                            p * 4096 + (rq * 4 + rr + 1) * 512],
                                    start=True, stop=True,
                                    tile_position=(g, 0),
                                    skip_group_check=True)
                            nc.scalar.activation(st[:], ps[:], AF.Sigmoid)
                            nc.sync.dma_start(
                                out4[m * 128:(m + 1) * 128,
                                     rq * 4:(rq + 1) * 4, p, :],
                                st.rearrange("m (r j) -> m r j", j=512))

    nc.compile()
    return nc


def _get_program():
    if "nc" not in _COMPILED:
        _COMPILED["nc"] = _build_program()
    return _COMPILED["nc"]


def _prep_inputs(x, edge_index, W1, b1, W2, b2):
    x = np.asarray(x, np.float32)
    ei = np.asarray(edge_index)
    src = ei[0].astype(np.int64)
    dst = ei[1].astype(np.int64)

    deg = (np.bincount(dst, minlength=N) + 1).astype(np.float32)
    degt = np.ascontiguousarray(deg.reshape(NCH, 128).T)           # [128, 64]

    xw = np.zeros((128, XW_COLS), dtype=ml_dtypes.bfloat16)
    xw[:, 0:HID] = np.asarray(W1, np.float32).astype(ml_dtypes.bfloat16)
    xw[0:HID, HID:2 * HID] = (
        np.asarray(W2, np.float32).astype(ml_dtypes.bfloat16))
    xw[:, 2 * HID:] = x.T.astype(ml_dtypes.bfloat16)

    b1c = np.asarray(b1, np.float32).reshape(HID)
    b2c = np.asarray(b2, np.float32).reshape(HID)

    in_maps = []
    for i in range(CORES):
        lo = i * DSH
        sel = (dst >= lo) & (dst < lo + DSH)
        flat = src[sel] * DSH + (dst[sel] - lo)
        cnt = np.bincount(flat, minlength=N * DSH).reshape(N, DSH)
        cnt[np.arange(lo, lo + DSH), np.arange(DSH)] += 1          # + I shard
        # SBUF layout: partition p holds src rows {c*128+p}, free = c*DSH + d
        A8 = np.ascontiguousarray(
            cnt.reshape(NCH, 128, DSH).transpose(1, 0, 2)
        ).astype(ml_dtypes.float8_e4m3).reshape(128, NCH * DSH)

        c32 = np.zeros((128, C_COLS), dtype=np.float32)
        c32[:, C_DEGT:C_DEGT + NCH] = degt
        c32[:, C_DEGSH:C_DEGSH + KSH] = deg[lo:lo + DSH].reshape(KSH, 128).T
        degb = np.broadcast_to(deg[lo:lo + DSH][None, :], (HID, DSH))
        c32[:, C_DEGB:C_DEGB + 512] = (
            degb.reshape(HID, 2, 512).transpose(1, 0, 2).reshape(128, 512))
        c32[0:HID, C_B1] = b1c
        c32[0:HID, C_B2] = b2c
        # rsqrt pipeline runs over every deg column; keep the b columns out
        # of it but the whole c32 tile must be finite for the Newton step.
        in_maps.append({"xw": xw, "A": A8, "c32": c32})
    return in_maps


def kernel(x, edge_index, W1, b1, W2, b2, _trace=False, _trace_kwargs=None):
    nc = _get_program()
    in_maps = _prep_inputs(x, edge_index, W1, b1, W2, b2)
    res = run_bass_kernel_spmd(nc, in_maps, core_ids=list(range(CORES)),
                               trace=_trace, **(_trace_kwargs or {}))
    out = np.concatenate([res.results[i]["out"] for i in range(CORES)], axis=0)
    if _trace:
        kernel._last_results = res
    return out.astype(np.float32)
